# revision 4
# baseline (speedup 1.0000x reference)
"""Trainium2 Bass kernel for KANPolyLayer:
    y[b,o] = sum_{i,p} x[b,i]^p * coeffs[o,i,p] + bias[o],  p = 0..4

Math: y = sum_{p=1..4} (x^p) @ C_p^T + (bias + colsum(C_0)), with
C_p = coeffs[:, :, p].  The p=0 plane and bias are folded on the host
(cheap O(out_dim*in_dim) reduction + broadcast add on gather); the
device does 4 accumulated GEMM planes in bf16 with powers computed
on-chip by the vector engine (x^2 = x*x, x^3 = x^2*x, x^4 = x^2*x^2).

Per-core schedule: everything is SBUF-resident (no tile rings).
Inputs stream in as a handful of large-packet chunked DMAs on the two
HWDGE queues (Sync: x, Scalar: coeffs), k=0 slices first and sized so
every (k,p) plane lands ~2us+ before its matmuls.  Coefficients use a
k-major DRAM layout ([ki, k*2048 + (p-1)*512 + o']) so bulk chunks
have 4KB+ rows (DMA packet size == row size; 1KB rows run ~40GB/s vs
~350GB/s at 6KB).  A warmup burst of garbage matmuls with no input
dependencies keeps the PE busy from the moment the engines start, so
the HAM clock-gate reaches 2.4 GHz just as the real stream begins.
All 8 (o-tile, b-half) output groups accumulate concurrently in the 8
PSUM banks; the trailing 2 k-planes are emitted group-contiguous so
each group's PSUM->SBUF copy (DVE) and output DMA overlap the
remaining matmul stream.  The kernel computes yT = [o, b]; the host
transposes and adds the folded bias row.

Sharding (8 cores): 4 batch groups x 2 out-dim groups.
  core c -> (bg, og) = (c // 2, c % 2)
Each core computes a disjoint (512 x 1024) block of yT; host gathers.
"""

from contextlib import ExitStack

import numpy as np
import ml_dtypes

import concourse.bacc as bacc
import concourse.bass as bass
import concourse.bass_utils as bass_utils
import concourse.mybir as mybir
import concourse.tile as tile
from concourse.bass_utils import run_bass_kernel_spmd

F32 = mybir.dt.float32
BF16 = mybir.dt.bfloat16
NP_BF16 = ml_dtypes.bfloat16

B, I, O = 4096, 1024, 1024  # batch, in_dim, out_dim
BW, OW = 4, 2               # batch groups x out-dim groups (8 cores)
BS, OS = B // BW, O // OW   # per-core batch (1024) and out (512)
NK = I // 128               # contraction k-tiles (8)
NT = OS // 128              # o-tiles (4)
NH = BS // 512              # b-halves (2)
NTAIL = 2                   # trailing k-planes emitted group-contiguous
NWARM = 30                  # warmup matmuls (N=128, cold ~107ns each)

_CACHE: dict = {}

# The NEFF epilogue injected by the backend zeroes the whole semaphore
# file one EVENT_SEMAPHORE per slot (~6us for 256 slots).  Capping the
# allocatable semaphore count shrinks that sweep; fall back to the
# default if the capped compile fails.
_MAX_SEM_ARG = "--max-sem-num=64"
_orig_get_walrus_args = bass_utils.get_walrus_args


def _patched_get_walrus_args(*a, **k):
    return [*_orig_get_walrus_args(*a, **k), _MAX_SEM_ARG]


def _build():
    nc = bacc.Bacc("TRN2", target_bir_lowering=False, debug=False, num_devices=8)

    # xt[ki, k*1024 + h*512 + b'] = x[bg*1024 + h*512 + b', k*128 + ki]
    xt = nc.dram_tensor("xt", [128, NK * BS], BF16, kind="ExternalInput")
    # ctk[ki, k*2048 + (p-1)*512 + o'] = coeffs[og*512 + o', k*128 + ki, p]
    ctk = nc.dram_tensor("ctk", [128, NK * 4 * OS], BF16, kind="ExternalInput")
    yt = nc.dram_tensor("yt", [OS, BS], F32, kind="ExternalOutput")  # [o, b]

    with tile.TileContext(nc) as tc, ExitStack() as ctx:
        cons = ctx.enter_context(tc.tile_pool(name="cons", bufs=1))
        xpool = ctx.enter_context(tc.tile_pool(name="x", bufs=1))
        cpool = ctx.enter_context(tc.tile_pool(name="coef", bufs=1))
        ppool = ctx.enter_context(tc.tile_pool(name="pow", bufs=1))
        opool = ctx.enter_context(tc.tile_pool(name="out", bufs=1))
        pspool = ctx.enter_context(
            tc.tile_pool(name="ps", bufs=8, space=bass.MemorySpace.PSUM)
        )

        # 8 concurrent accumulation groups: (o-tile, b-half) -> one PSUM bank
        ps = {}
        for ot in range(NT):
            for h in range(NH):
                ps[(ot, h)] = pspool.tile(
                    [128, 512], F32, tag="ps", name=f"ps_{ot}_{h}"
                )

        # PE warmup: garbage matmuls on a memset tile, issued with no input
        # dependencies, sized to span until the first input chunks land so
        # the HAM activity window stays continuously busy and the clock
        # reaches 2.4 GHz right as the real stream begins.
        wz = cons.tile([128, 128], BF16)
        nc.vector.memset(wz[:], 0.0)
        for _ in range(NWARM):
            nc.tensor.matmul(
                ps[(0, 0)][:, 0:128], wz[:], wz[:], start=True, stop=True,
                skip_group_check=True,
            )

        # ---- input DMAs: two parallel HWDGE issue queues ----
        # Sync queue: x in 3 chunks (k0 | k1-3 | k4-7)
        xh = xpool.tile([128, 1024], BF16, tag="xh", name="xh")
        xm = xpool.tile([128, 3072], BF16, tag="xm", name="xm")
        xe = xpool.tile([128, 4096], BF16, tag="xe", name="xe")
        nc.sync.dma_start(xh[:], xt[:, 0:1024])
        nc.sync.dma_start(xm[:], xt[:, 1024:4096])
        nc.sync.dma_start(xe[:], xt[:, 4096:8192])

        def xs(k, h=None):
            """x^1 slice for k-tile k (both halves, or one half h)."""
            t, base = (xh, 0) if k == 0 else (xm, 1024) if k < 4 else (xe, 4096)
            off = k * 1024 - base + (0 if h is None else h * 512)
            return t[:, off:off + (1024 if h is None else 512)]

        # Scalar queue: coefficients, k-major layout; k0 split fine for
        # latency, bulk in large-row chunks.
        ct = cpool.tile([128, NK * 4 * OS], BF16, tag="ct", name="ct")
        for lo, hi in [(0, 512), (512, 1536), (1536, 2048),      # k0: p1 | p2,p3 | p4
                       (2048, 4096), (4096, 8192),               # k1 | k2,k3
                       (8192, 16384)]:                           # k4-7
            nc.scalar.dma_start(ct[:, lo:hi], ctk[:, lo:hi])

        # ---- powers on DVE, per k-tile ----
        p2 = ppool.tile([128, NK * BS], BF16, tag="p2", name="p2")
        p3 = ppool.tile([128, NK * BS], BF16, tag="p3", name="p3")
        p4 = ppool.tile([128, NK * BS], BF16, tag="p4", name="p4")

        def pows(p, k, h):
            if p == 1:
                return xs(k, h)
            t = (None, None, p2, p3, p4)[p]
            off = k * 1024 + h * 512
            return t[:, off:off + 512]

        for k in range(NK):
            s = xs(k)
            d0 = k * 1024
            nc.vector.tensor_mul(p2[:, d0:d0 + 1024], s, s)
            nc.vector.tensor_mul(p3[:, d0:d0 + 1024], p2[:, d0:d0 + 1024], s)
            nc.vector.tensor_mul(p4[:, d0:d0 + 1024], p2[:, d0:d0 + 1024],
                                 p2[:, d0:d0 + 1024])

        def mm(k, p, ot, h, start, stop):
            base = k * 2048 + (p - 1) * 512 + ot * 128
            w = ct[:, base:base + 128]
            nc.tensor.matmul(ps[(ot, h)], w, pows(p, k, h), start=start, stop=stop)

        # main stream: k-major, all 8 groups accumulate per (k, p) plane
        for k in range(NK - NTAIL):
            for p in range(1, 5):
                for ot in range(NT):
                    for h in range(NH):
                        mm(k, p, ot, h, start=(k == 0 and p == 1), stop=False)

        # trailing k-planes group-contiguous: groups finish ~1.8us apart so
        # each PSUM->SBUF copy + output DMA overlaps the matmul stream
        for gi, (ot, h) in enumerate([(ot, h) for ot in range(NT) for h in range(NH)]):
            for k in range(NK - NTAIL, NK):
                for p in range(1, 5):
                    mm(k, p, ot, h, start=False, stop=(k == NK - 1 and p == 4))
            o_sb = opool.tile([128, 512], F32, tag=f"o{gi}", name=f"o_{ot}_{h}")
            nc.vector.tensor_copy(o_sb[:], ps[(ot, h)][:])
            eng = nc.sync if gi % 2 == 0 else nc.scalar
            eng.dma_start(
                yt[ot * 128:(ot + 1) * 128, h * 512:(h + 1) * 512], o_sb[:]
            )

    nc.compile()
    return nc


def _get_nc():
    if "nc" not in _CACHE:
        _CACHE["nc"] = _build()
    return _CACHE["nc"]


def _pack_x(xs_block):
    # [1024b, 1024i] -> [ki, k*1024 + h*512 + b']
    a = xs_block.astype(NP_BF16)
    return np.ascontiguousarray(
        a.reshape(NH, 512, NK, 128).transpose(3, 2, 0, 1).reshape(128, NK * BS)
    )


def _pack_c(c_block):
    # [512o', 1024i, 4p] -> [ki, k*2048 + (p-1)*512 + o']
    a = c_block.astype(NP_BF16)
    return np.ascontiguousarray(
        a.transpose(1, 2, 0).reshape(NK, 128, 4, OS).transpose(1, 0, 2, 3)
        .reshape(128, NK * 4 * OS)
    )


def _make_in_maps(x, coeffs):
    x = np.asarray(x, dtype=np.float32)
    coeffs = np.asarray(coeffs, dtype=np.float32)
    xts = [_pack_x(x[bg * BS:(bg + 1) * BS, :]) for bg in range(BW)]
    cts = [_pack_c(coeffs[og * OS:(og + 1) * OS, :, 1:5]) for og in range(OW)]
    in_maps = []
    for c in range(BW * OW):
        bg, og = c // OW, c % OW
        in_maps.append({"xt": xts[bg], "ctk": cts[og]})
    return in_maps


def _gather(results, base):
    y = np.empty((B, O), dtype=np.float32)
    for c, res in enumerate(results):
        bg, og = c // OW, c % OW
        y[bg * BS:(bg + 1) * BS, og * OS:(og + 1) * OS] = (
            res["yt"].T + base[og * OS:(og + 1) * OS]
        )
    return y


def run(x, coeffs, bias, trace=False, **trace_kwargs):
    nc = _get_nc()
    in_maps = _make_in_maps(x, coeffs)
    # p=0 plane (x^0 == 1) and bias folded on host:
    base = (
        np.asarray(coeffs, dtype=np.float32)[:, :, 0].sum(axis=1)
        + np.asarray(bias, dtype=np.float32)[0]
    )
    bass_utils.get_walrus_args = _patched_get_walrus_args
    try:
        br = run_bass_kernel_spmd(
            nc, in_maps, list(range(BW * OW)), trace=trace, **trace_kwargs
        )
    except Exception:
        # capped-semaphore compile failed: retry with default walrus args
        bass_utils.get_walrus_args = _orig_get_walrus_args
        br = run_bass_kernel_spmd(
            nc, in_maps, list(range(BW * OW)), trace=trace, **trace_kwargs
        )
    finally:
        bass_utils.get_walrus_args = _orig_get_walrus_args
    return _gather(br.results, base), br


def kernel(x, coeffs, bias):
    out, _ = run(x, coeffs, bias)
    return out


# revision 7
# speedup vs baseline: 1.0535x; 1.0535x over previous
"""Trainium2 Bass kernel for KANPolyLayer:
    y[b,o] = sum_{i,p} x[b,i]^p * coeffs[o,i,p] + bias[o],  p = 0..4

Math: y = sum_{p=1..4} (x^p) @ C_p^T + (bias + colsum(C_0)), with
C_p = coeffs[:, :, p].  The p=0 plane and bias are folded on the host
(cheap O(out_dim*in_dim) reduction + broadcast add on gather); the
device does 4 accumulated GEMM planes in bf16 with powers computed
on-chip by the vector engine (x^2 = x*x, x^3 = x^2*x, x^4 = x^2*x^2).

Per-core schedule: everything is SBUF-resident (no tile rings).
Inputs stream in as a handful of large-packet chunked DMAs on the two
HWDGE queues (Sync: x, Scalar: coeffs), k=0 slices first and sized so
every (k,p) plane lands ~2us+ before its matmuls.  Coefficients use a
k-major DRAM layout ([ki, k*2048 + (p-1)*512 + o']) so bulk chunks
have 4KB+ rows (DMA packet size == row size; 1KB rows run ~40GB/s vs
~350GB/s at 6KB).  A warmup burst of garbage matmuls with no input
dependencies keeps the PE busy from the moment the engines start, so
the HAM clock-gate reaches 2.4 GHz just as the real stream begins.
All 8 (o-tile, b-half) output groups accumulate concurrently in the 8
PSUM banks; the trailing 2 k-planes are emitted group-contiguous so
each group's PSUM->SBUF copy (DVE) and output DMA overlap the
remaining matmul stream.  The kernel computes yT = [o, b]; the host
transposes and adds the folded bias row.

Sharding (8 cores): 4 batch groups x 2 out-dim groups.
  core c -> (bg, og) = (c // 2, c % 2)
Each core computes a disjoint (512 x 1024) block of yT; host gathers.
"""

from contextlib import ExitStack

import numpy as np
import ml_dtypes

import concourse.bacc as bacc
import concourse.bass as bass
import concourse.bass_utils as bass_utils
import concourse.mybir as mybir
import concourse.tile as tile
from concourse.bass_utils import run_bass_kernel_spmd

F32 = mybir.dt.float32
BF16 = mybir.dt.bfloat16
NP_BF16 = ml_dtypes.bfloat16

B, I, O = 4096, 1024, 1024  # batch, in_dim, out_dim
BW, OW = 4, 2               # batch groups x out-dim groups (8 cores)
BS, OS = B // BW, O // OW   # per-core batch (1024) and out (512)
NK = I // 128               # contraction k-tiles (8)
NT = OS // 128              # o-tiles (4)
NH = BS // 512              # b-halves (2)
NTAIL = 2                   # trailing k-planes emitted group-contiguous
NWARM = 30                  # warmup matmuls (N=128, cold ~107ns each)

_CACHE: dict = {}


def _build():
    nc = bacc.Bacc("TRN2", target_bir_lowering=False, debug=False, num_devices=8)

    # xt[ki, k*1024 + h*512 + b'] = x[bg*1024 + h*512 + b', k*128 + ki]
    xt = nc.dram_tensor("xt", [128, NK * BS], BF16, kind="ExternalInput")
    # ctk[ki, k*2048 + (p-1)*512 + o'] = coeffs[og*512 + o', k*128 + ki, p]
    ctk = nc.dram_tensor("ctk", [128, NK * 4 * OS], BF16, kind="ExternalInput")
    yt = nc.dram_tensor("yt", [OS, BS], F32, kind="ExternalOutput")  # [o, b]

    with tile.TileContext(nc) as tc, ExitStack() as ctx:
        cons = ctx.enter_context(tc.tile_pool(name="cons", bufs=1))
        xpool = ctx.enter_context(tc.tile_pool(name="x", bufs=1))
        cpool = ctx.enter_context(tc.tile_pool(name="coef", bufs=1))
        ppool = ctx.enter_context(tc.tile_pool(name="pow", bufs=1))
        opool = ctx.enter_context(tc.tile_pool(name="out", bufs=1))
        pspool = ctx.enter_context(
            tc.tile_pool(name="ps", bufs=8, space=bass.MemorySpace.PSUM)
        )

        # 8 concurrent accumulation groups: (o-tile, b-half) -> one PSUM bank
        ps = {}
        for ot in range(NT):
            for h in range(NH):
                ps[(ot, h)] = pspool.tile(
                    [128, 512], F32, tag="ps", name=f"ps_{ot}_{h}"
                )

        # PE warmup: garbage matmuls on a memset tile, issued with no input
        # dependencies, sized to span until the first input chunks land so
        # the HAM activity window stays continuously busy and the clock
        # reaches 2.4 GHz right as the real stream begins.
        wz = cons.tile([128, 128], BF16)
        nc.vector.memset(wz[:], 0.0)
        for _ in range(NWARM):
            nc.tensor.matmul(
                ps[(0, 0)][:, 0:128], wz[:], wz[:], start=True, stop=True,
                skip_group_check=True,
            )

        # ---- input DMAs: two parallel HWDGE issue queues ----
        # DMA packet rate is ~constant per packet (packet == row), so BW
        # scales with row size; the scalar queue also starts ~0.9us after
        # sync.  Order chunks so every plane lands ~1.5us+ ahead of its
        # consumption: sync carries x + the k0 p2/p4 heads, scalar the k0
        # p1/p3 heads + the big k-major coefficient bulk.
        xh = xpool.tile([128, 1024], BF16, tag="xh", name="xh")
        xm = xpool.tile([128, 3072], BF16, tag="xm", name="xm")
        xe = xpool.tile([128, 4096], BF16, tag="xe", name="xe")
        ct = cpool.tile([128, NK * 4 * OS], BF16, tag="ct", name="ct")

        nc.sync.dma_start(xh[:], xt[:, 0:1024])                   # k0 x
        nc.scalar.dma_start(ct[:, 0:512], ctk[:, 0:512])          # k0 p1
        nc.sync.dma_start(ct[:, 512:1024], ctk[:, 512:1024])      # k0 p2
        nc.scalar.dma_start(ct[:, 1024:1536], ctk[:, 1024:1536])  # k0 p3
        nc.sync.dma_start(ct[:, 1536:2048], ctk[:, 1536:2048])    # k0 p4
        nc.scalar.dma_start(ct[:, 2048:4096], ctk[:, 2048:4096])  # k1
        nc.sync.dma_start(xm[:], xt[:, 1024:4096])                # k1-3 x
        nc.scalar.dma_start(ct[:, 4096:8192], ctk[:, 4096:8192])  # k2,k3
        nc.sync.dma_start(xe[:], xt[:, 4096:8192])                # k4-7 x
        nc.scalar.dma_start(ct[:, 8192:16384], ctk[:, 8192:16384])  # k4-7

        def xs(k, h=None):
            """x^1 slice for k-tile k (both halves, or one half h)."""
            t, base = (xh, 0) if k == 0 else (xm, 1024) if k < 4 else (xe, 4096)
            off = k * 1024 - base + (0 if h is None else h * 512)
            return t[:, off:off + (1024 if h is None else 512)]

        # ---- powers on DVE, per k-tile ----
        p2 = ppool.tile([128, NK * BS], BF16, tag="p2", name="p2")
        p3 = ppool.tile([128, NK * BS], BF16, tag="p3", name="p3")
        p4 = ppool.tile([128, NK * BS], BF16, tag="p4", name="p4")

        def pows(p, k, h):
            if p == 1:
                return xs(k, h)
            t = (None, None, p2, p3, p4)[p]
            off = k * 1024 + h * 512
            return t[:, off:off + 512]

        for k in range(NK):
            s = xs(k)
            d0 = k * 1024
            nc.vector.tensor_mul(p2[:, d0:d0 + 1024], s, s)
            nc.vector.tensor_mul(p3[:, d0:d0 + 1024], p2[:, d0:d0 + 1024], s)
            nc.vector.tensor_mul(p4[:, d0:d0 + 1024], p2[:, d0:d0 + 1024],
                                 p2[:, d0:d0 + 1024])

        def mm(k, p, ot, h, start, stop):
            base = k * 2048 + (p - 1) * 512 + ot * 128
            w = ct[:, base:base + 128]
            nc.tensor.matmul(ps[(ot, h)], w, pows(p, k, h), start=start, stop=stop)

        # main stream: k-major, all 8 groups accumulate per (k, p) plane
        for k in range(NK - NTAIL):
            for p in range(1, 5):
                for ot in range(NT):
                    for h in range(NH):
                        mm(k, p, ot, h, start=(k == 0 and p == 1), stop=False)

        # trailing k-planes group-contiguous: groups finish ~1.8us apart so
        # each PSUM->SBUF copy + output DMA overlaps the matmul stream
        for gi, (ot, h) in enumerate([(ot, h) for ot in range(NT) for h in range(NH)]):
            for k in range(NK - NTAIL, NK):
                for p in range(1, 5):
                    mm(k, p, ot, h, start=False, stop=(k == NK - 1 and p == 4))
            o_sb = opool.tile([128, 512], F32, tag=f"o{gi}", name=f"o_{ot}_{h}")
            nc.vector.tensor_copy(o_sb[:], ps[(ot, h)][:])
            eng = nc.sync if gi % 2 == 0 else nc.scalar
            eng.dma_start(
                yt[ot * 128:(ot + 1) * 128, h * 512:(h + 1) * 512], o_sb[:]
            )

    nc.compile()
    return nc


def _get_nc():
    if "nc" not in _CACHE:
        _CACHE["nc"] = _build()
    return _CACHE["nc"]


def _pack_x(xs_block):
    # [1024b, 1024i] -> [ki, k*1024 + h*512 + b']
    a = xs_block.astype(NP_BF16)
    return np.ascontiguousarray(
        a.reshape(NH, 512, NK, 128).transpose(3, 2, 0, 1).reshape(128, NK * BS)
    )


def _pack_c(c_block):
    # [512o', 1024i, 4p] -> [ki, k*2048 + (p-1)*512 + o']
    a = c_block.astype(NP_BF16)
    return np.ascontiguousarray(
        a.transpose(1, 2, 0).reshape(NK, 128, 4, OS).transpose(1, 0, 2, 3)
        .reshape(128, NK * 4 * OS)
    )


def _make_in_maps(x, coeffs):
    x = np.asarray(x, dtype=np.float32)
    coeffs = np.asarray(coeffs, dtype=np.float32)
    xts = [_pack_x(x[bg * BS:(bg + 1) * BS, :]) for bg in range(BW)]
    cts = [_pack_c(coeffs[og * OS:(og + 1) * OS, :, 1:5]) for og in range(OW)]
    in_maps = []
    for c in range(BW * OW):
        bg, og = c // OW, c % OW
        in_maps.append({"xt": xts[bg], "ctk": cts[og]})
    return in_maps


def _gather(results, base):
    y = np.empty((B, O), dtype=np.float32)
    for c, res in enumerate(results):
        bg, og = c // OW, c % OW
        y[bg * BS:(bg + 1) * BS, og * OS:(og + 1) * OS] = (
            res["yt"].T + base[og * OS:(og + 1) * OS]
        )
    return y


def run(x, coeffs, bias, trace=False, **trace_kwargs):
    nc = _get_nc()
    in_maps = _make_in_maps(x, coeffs)
    # p=0 plane (x^0 == 1) and bias folded on host:
    base = (
        np.asarray(coeffs, dtype=np.float32)[:, :, 0].sum(axis=1)
        + np.asarray(bias, dtype=np.float32)[0]
    )
    br = run_bass_kernel_spmd(
        nc, in_maps, list(range(BW * OW)), trace=trace, **trace_kwargs
    )
    return _gather(br.results, base), br


def kernel(x, coeffs, bias):
    out, _ = run(x, coeffs, bias)
    return out
